# revision 1
# baseline (speedup 1.0000x reference)
"""Trainium2 Bass kernel for nn_Encoder (Tacotron2-style encoder):
3x(Conv1d K=5 + BatchNorm(eval) + ReLU) -> bidirectional LSTM (H=256/dir)
with zoneout(p=0.1, eval).

Sharding: 8 cores = 2 directions x 4 batch-groups (8 samples each).
The backward direction runs the SAME program on time-reversed input with
tap-flipped conv weights; the host reverses its output back.

Per-core pipeline:
  A small conv prologue covers t<134 and feeds the first 125-step
  x-projection block so the recurrence can start almost immediately.
  The remaining conv work (BN scale folded into the fp16 weights) +
  x-projections are chopped into ~50-250ns work items, queued in
  time-order (125-step chunks), and paced into the engine gaps of the
  LSTM recurrence, which runs as a single 8-sample chain.  The
  recurrence wall time is 1000 x the per-step dependency cycle, which
  is minimized to  u -> 16 Whh@u matmuls -> fused sigmoid over all 4
  gates (g pre-doubled so tanh(g)=2*sig(2g)-1) -> 3 DVE ops -> tanh
  -> u  by exploiting zoneout linearity:
  Whh@h(t) = (P*Whh)@h(t-1) + Whh@u(t), so the P-part and the h/c
  state updates run off the critical path (h on DVE, c on Pool).
"""
import os
from collections import deque

import numpy as np

import concourse.bacc as bacc
import concourse.tile as tile
import concourse.mybir as mybir
from concourse.bass_utils import run_bass_kernel_spmd
from concourse.masks import make_identity

F32 = mybir.dt.float32
F32R = mybir.dt.float32r
F16 = mybir.dt.float16
AF = mybir.ActivationFunctionType
OP = mybir.AluOpType

B, C_IN, T = 32, 80, 1000
C, H, K = 512, 256, 5
BL = 8                       # samples per core
BH = BL // 2                 # samples per chain
TP = T + 4                   # padded time
P_ZO = 0.1                   # zoneout keep prob
Q_ZO = 1.0 - P_ZO
BN_EPS = 1e-5
RB = 25                      # steps per ring/out group
NJJ = 8                      # xproj 125-step blocks
CW = 136                     # conv chunk tile width (133 used)

_CACHE = {}


def _build():
    nc = bacc.Bacc("TRN2", target_bir_lowering=False, debug=False,
                   num_devices=8)

    x_d = nc.dram_tensor("x", [C_IN, BL, TP], F16, kind="ExternalInput")
    w0_d = nc.dram_tensor("w0", [C_IN, K, C], F16, kind="ExternalInput")
    w1_d = nc.dram_tensor("w1", [128, 4, K, C], F16, kind="ExternalInput")
    w2_d = nc.dram_tensor("w2", [128, 4, K, C], F16, kind="ExternalInput")
    bn_d = nc.dram_tensor("bn", [128, 3, 2, 4], F32, kind="ExternalInput")
    wih_d = nc.dram_tensor("wih", [128, 4, 4 * H], F16, kind="ExternalInput")
    bg_d = nc.dram_tensor("bg", [1, 4 * H], F32, kind="ExternalInput")
    whh_d = nc.dram_tensor("whh", [128, 2, 4 * H], F16, kind="ExternalInput")
    whhp_d = nc.dram_tensor("whhp", [128, 2, 4 * H], F16,
                            kind="ExternalInput")
    out_d = nc.dram_tensor("out", [T // RB, 128, RB * 2 * BL], F16,
                           kind="ExternalOutput")

    with tile.TileContext(nc) as tc:
        with (
            tc.tile_pool(name="const", bufs=1) as cpool,
            tc.tile_pool(name="blk", bufs=3) as blk,
            tc.tile_pool(name="cps", bufs=2, space="PSUM") as cps,
            tc.tile_pool(name="xps", bufs=2, space="PSUM") as xps,
            tc.tile_pool(name="xsb", bufs=1) as xsb,
            tc.tile_pool(name="gps", bufs=2, space="PSUM") as gps,
            tc.tile_pool(name="step", bufs=3) as sp,
            tc.tile_pool(name="ring", bufs=3) as rp,
            tc.tile_pool(name="dram", bufs=1, space="DRAM") as dp,
        ):
            # per-125-step xproj staging buffers in HBM, layout [t,m,p,b]
            xpt = [dp.tile([125, 8, 128, BL], F16, name=f"xp{j}")
                   for j in range(NJJ)]

            # ---- constants / weights in SBUF ----
            x_sb = cpool.tile([C_IN, BL, TP], F16)
            nc.sync.dma_start(x_sb[:], x_d[:])
            w0 = cpool.tile([C_IN, K, C], F16)
            nc.sync.dma_start(w0[:], w0_d[:])
            w1 = cpool.tile([128, 4, K, C], F16, tag="bigw0")
            nc.sync.dma_start(w1[:], w1_d[:])
            w2 = cpool.tile([128, 4, K, C], F16, tag="bigw1")
            nc.sync.dma_start(w2[:], w2_d[:])
            bn = cpool.tile([128, 3, 2, 4], F32)
            nc.sync.dma_start(bn[:], bn_d[:])
            wih = cpool.tile([128, 4, 4 * H], F16)
            nc.sync.dma_start(wih[:], wih_d[:])
            whh = cpool.tile([128, 2, 4 * H], F16)
            nc.sync.dma_start(whh[:], whh_d[:])
            whhp = cpool.tile([128, 2, 4 * H], F16, tag="whhp")
            nc.sync.dma_start(whhp[:], whhp_d[:])
            bgate_f = sp.tile([1, 4 * H], F32, name="bgf", tag="bgf")
            nc.sync.dma_start(bgate_f[:], bg_d[:])
            bgate = cpool.tile([1, 4 * H], F32R)
            nc.vector.tensor_copy(bgate[:], bgate_f[:])
            ones_f = sp.tile([1, 128], F32, name="onesf", tag="onesf")
            nc.gpsimd.memset(ones_f[:], 1.0)
            ones = cpool.tile([1, 128], F32R)
            nc.vector.tensor_copy(ones[:], ones_f[:])
            hzero = cpool.tile([128, 2, BL], F16)
            nc.gpsimd.memset(hzero[:], 0.0)
            czero = cpool.tile([128, 2, BL], F32)
            nc.gpsimd.memset(czero[:], 0.0)
            pconst = cpool.tile([128, 2, BL], F32, tag="pconst")
            nc.gpsimd.memset(pconst[:], P_ZO)
            ident = cpool.tile([128, 128], F16)
            make_identity(nc, ident[:])

            # =========== conv prologue (eager): t < 129-2l ===========
            # quarter tile col c <-> t = c - 6; feeds ONLY xproj block 0
            # (l2 t<125) -- queued chunks recompute their own halos.
            prevq = None
            for l in range(3):
                otq = blk.tile([128, 4, BL, 144], F16, name=f"q{l}",
                               tag="blkq", bufs=2)
                nc.gpsimd.memset(otq[:, :, :, 0:6], 0.0)
                n = 129 - 2 * l
                nm = 4 if l > 0 else 1
                w_l = (w0, w1, w2)[l]
                for m in range(4):
                    for b in range(BL):
                        ps = cps.tile([128, 506], F32, name="cps", tag="cps")
                        first = True
                        for q in range(nm):
                            for k in range(K):
                                if l == 0:
                                    lhsT = w_l[:, k, 128 * m:128 * (m + 1)]
                                    rhs = x_sb[:, b, k:k + n]
                                else:
                                    lhsT = w_l[:, q, k, 128 * m:128 * (m + 1)]
                                    rhs = prevq[:, q, b, 4 + k:4 + k + n]
                                nc.tensor.matmul(ps[:, 0:n], lhsT, rhs,
                                                 start=first,
                                                 stop=(q == nm - 1 and
                                                       k == K - 1))
                                first = False
                        # BN scale folded into weights; alternate the
                        # bias+ReLU epilogue across ACT/DVE to overlap
                        if b % 2 == 0:
                            nc.scalar.activation(
                                otq[:, m, b, 6:6 + n], ps[:, 0:n],
                                AF.Relu, bias=bn[:, l, 1, m:m + 1],
                                scale=bn[:, l, 0, m:m + 1])
                        else:
                            nc.vector.tensor_scalar(
                                otq[:, m, b, 6:6 + n], ps[:, 0:n],
                                bn[:, l, 1, m:m + 1], 0.0,
                                OP.add, OP.max)
                prevq = otq

            def xproj_emit(j, feat, c0, copy_eng):
                """x-projections for steps 125j..125j+124 from feat tile
                (cols c0..c0+125), staged to xpt[j]."""
                stg = xsb.tile([125, 8, 128, BL], F16, name="stg", tag="stg")
                for b in range(BL):
                    for nn2 in range(2):
                        ps = xps.tile([125, 512], F32, name="xps", tag="xps")
                        for q in range(4):
                            yield 220, lambda b=b, nn2=nn2, ps=ps, q=q: \
                                nc.tensor.matmul(
                                    ps[:],
                                    feat[:, q, b, c0:c0 + 125],
                                    wih[:, q, 512 * nn2:512 * (nn2 + 1)],
                                    start=(q == 0), stop=False)
                        yield 220, lambda b=b, nn2=nn2, ps=ps: \
                            nc.tensor.matmul(
                                ps[:], ones[:, 0:125],
                                bgate[:, 512 * nn2:512 * (nn2 + 1)],
                                start=False, stop=True)
                        yield 810, lambda b=b, nn2=nn2, ps=ps: copy_eng(
                            stg[:, 4 * nn2:4 * (nn2 + 1), :, b],
                            ps[:].rearrange("t (m p) -> t m p", p=128))
                yield 600, lambda: nc.sync.dma_start(xpt[j][:], stg[:])

            # prologue xproj block 0 (eager, copies alternate ACT/DVE)
            _pcnt = [0]

            def _pro_copy(o, i):
                _pcnt[0] += 1
                if _pcnt[0] % 2:
                    nc.scalar.activation(o, i, AF.Copy)
                else:
                    nc.vector.tensor_copy(o, i)

            for cost, fn in xproj_emit(0, prevq, 6, _pro_copy):
                fn()

            # =========== queued conv chunks j=1..7 + xproj ===========
            # chunk j: l0 t in [125j-4, 125j+129), l1 [125j-2, 125j+127),
            # l2 [125j, 125j+125); tile col c <-> t = (125j-4) + c.
            # t >= 1000 halo cols are memset to 0 (zero padding).
            work_q = deque()
            marks = {0: True}
            total_cost = [0.0]

            def push(cost, fn):
                work_q.append((cost, fn))
                total_cost[0] += cost

            def run_one():
                cost, fn = work_q.popleft()
                fn()
                total_cost[0] -= cost
                return cost

            def pace(budget):
                while work_q and budget > 0.0:
                    budget -= run_one()

            def drain_mark(j):
                while not marks.get(j):
                    if not work_q:
                        raise RuntimeError(f"mark {j} never queued")
                    run_one()

            def conv_chunk_items(j, tiles):
                base = 125 * j - 4
                for l in range(3):
                    t_lo = base + 2 * l
                    n = min(t_lo + 133 - 4 * l, 1000) - t_lo
                    c_lo = t_lo - base
                    nm = 4 if l > 0 else 1
                    w_l = (w0, w1, w2)[l]

                    def mk_tile(l=l, t_lo=t_lo, n=n, c_lo=c_lo):
                        ot = blk.tile([128, 4, BL, CW], F16, name=f"ck{l}",
                                      tag="blk")
                        if t_lo + n >= 1000 and c_lo + n < CW:
                            nc.gpsimd.memset(ot[:, :, :, c_lo + n:CW], 0.0)
                        tiles[l] = ot
                    yield 50, mk_tile
                    for m in range(4):
                        for b in range(BL):
                            cell = {}

                            def mk_ps(cell=cell, n=n):
                                cell["ps"] = cps.tile([128, 506], F32,
                                                      name="cps", tag="cps")
                            yield 10, mk_ps
                            for q in range(nm):
                                for k in range(K):
                                    last = (q == nm - 1 and k == K - 1)

                                    def mm(l=l, m=m, b=b, q=q, k=k,
                                           cell=cell, n=n, c_lo=c_lo,
                                           t_lo=t_lo, w_l=w_l, last=last,
                                           first=(q == 0 and k == 0)):
                                        ps = cell["ps"]
                                        if l == 0:
                                            lhsT = w_l[:, k,
                                                       128 * m:128 * (m + 1)]
                                            rhs = x_sb[:, b,
                                                       t_lo + k:t_lo + k + n]
                                        else:
                                            lhsT = w_l[:, q, k,
                                                       128 * m:128 * (m + 1)]
                                            rhs = tiles[l - 1][
                                                :, q, b,
                                                c_lo - 2 + k:c_lo - 2 + k + n]
                                        nc.tensor.matmul(ps[:, 0:n], lhsT,
                                                         rhs, start=first,
                                                         stop=last)
                                    yield n * 0.42 + 3, mm
                            def ep_dve(l=l, m=m, b=b, cell=cell,
                                       c_lo=c_lo, n=n):
                                nc.vector.tensor_scalar(
                                    tiles[l][:, m, b, c_lo:c_lo + n],
                                    cell["ps"][:, 0:n],
                                    bn[:, l, 1, m:m + 1], 0.0,
                                    OP.add, OP.max)
                            yield 280, ep_dve

            for j in range(1, NJJ):
                tiles_j = {}
                for cost, fn in conv_chunk_items(j, tiles_j):
                    push(cost, fn)
                for cost, fn in xproj_emit(
                        j, _LateTile(tiles_j, 2), 4,
                        lambda o, i: nc.vector.tensor_copy(o, i)):
                    push(cost, fn)
                push(1, lambda j=j: marks.__setitem__(j, True))

            # ====== recurrence: single chain, split recurrent matmul ======
            # Zoneout linearity: h(t) = P*h(t-1) + u(t), so
            #   Whh@h(t) = (P*Whh)@h(t-1) + Whh@u(t).
            # The P-part (whhp) runs early, off the critical path; only the
            # 16 Whh@u matmuls sit between u(t) and sigma(t+1).
            xr_tiles = {}

            def get_xr(g):
                if g not in xr_tiles:
                    blkj = (g * RB) // 125
                    drain_mark(blkj)
                    xr = rp.tile([128, RB, 8, BL], F16, name="xr", tag="xr",
                                 bufs=3)
                    toff = g * RB - 125 * blkj
                    nc.sync.dma_start(
                        xr[:],
                        xpt[blkj][toff:toff + RB]
                        .rearrange("t m p b -> p t m b"))
                    xr_tiles[g] = xr
                return xr_tiles[g]

            pg_t = {}

            def mm_start(t, h_ap):
                """open pg(t): xr inject + (P*Whh)@h(t-2)-part (h_ap)."""
                if t >= T:
                    return
                g, s = t // RB, t % RB
                xr = get_xr(g)
                pg = gps.tile([128, 8, BL], F32, name="pg", tag="gps",
                              bufs=4)
                nc.tensor.matmul(pg[:], ident[:], xr[:, s, :, :],
                                 start=True, stop=False)
                if h_ap is not None:
                    for m in range(8):
                        for kc in range(2):
                            nc.tensor.matmul(
                                pg[:, m, :],
                                whhp[:, kc, 128 * m:128 * (m + 1)],
                                h_ap[:, kc, :],
                                start=False, stop=False)
                pg_t[t] = pg

            def mm_finish(t, u_ap):
                """close pg(t): Whh@u(t-1)-part."""
                pg = pg_t[t]
                for m in range(8):
                    for kc in range(2):
                        nc.tensor.matmul(
                            pg[:, m, :],
                            whh[:, kc, 128 * m:128 * (m + 1)],
                            u_ap[:, kc, :],
                            start=False, stop=(m == 7 and kc == 1))

            def close_pg0(t):
                """t=0: gates are xr only (h(-1)=0, u(-1)=0)."""
                pg = pg_t[t]
                nc.tensor.matmul(pg[:, 0, :], ident[:, 0:128],
                                 hzero[:, 0, :], start=False, stop=True)

            # elementwise step; m-blocks 0:2=i, 2:4=g(2x), 4:6=f, 6:8=o.
            #   tanh(g) = 2*sig(2g)-1:
            #   wv = (sig2g - 0.5)*sigi;  v2 = Q*sigf*c
            #   w  = 2Q*wv + v2 = Q*c2;   c' = P*c + w
            #   tc = tanh(w/Q);  u = Q*sigo*tc;  h' = P*h + u
            def elem_a(pg, c_ap, sfx=""):
                sall = sp.tile([128, 8, BL], F16, name="sall", tag="sall", bufs=5)
                nc.scalar.activation(sall[:], pg[:], AF.Sigmoid)
                wv = sp.tile([128, 2, BL], F16, name="wv", tag="wv", bufs=5)
                nc.vector.scalar_tensor_tensor(
                    wv[:], sall[:, 2:4, :], 0.5, sall[:, 0:2, :],
                    OP.subtract, OP.mult)
                v2 = sp.tile([128, 2, BL], F16, name="v2", tag="v2", bufs=5)
                nc.vector.scalar_tensor_tensor(
                    v2[:], sall[:, 4:6, :], Q_ZO, c_ap, OP.mult, OP.mult)
                w_t = sp.tile([128, 2, BL], F32, name="w", tag="w", bufs=5)
                nc.vector.scalar_tensor_tensor(
                    w_t[:], wv[:], 2.0 * Q_ZO, v2[:], OP.mult, OP.add)
                # c' = P*c + w on Pool (2 tensor_tensor ops: Pool rejects
                # TensorScalarPtr), freeing a DVE queue slot
                cp_ = sp.tile([128, 2, BL], F32, name="cp", tag="cp", bufs=5)
                nc.gpsimd.tensor_tensor(cp_[:], c_ap, pconst[:], OP.mult)
                c_new = sp.tile([128, 2, BL], F32, name="c", tag="c", bufs=5)
                nc.gpsimd.tensor_tensor(c_new[:], cp_[:], w_t[:], OP.add)
                tc2 = sp.tile([128, 2, BL], F16, name="tc2", tag="tc2", bufs=5)
                nc.scalar.activation(tc2[:], w_t[:], AF.Tanh,
                                     scale=1.0 / Q_ZO)
                u = sp.tile([128, 2, BL], F16, name="u", tag="u", bufs=5)
                nc.vector.scalar_tensor_tensor(
                    u[:], sall[:, 6:8, :], Q_ZO, tc2[:], OP.mult, OP.mult)
                return c_new[:], u[:]

            hring = None
            h_ap = hzero[:]
            c_ap = czero[:]
            PACE = float(os.environ.get("ENC_PACE", "1150"))

            mm_start(0, None)
            close_pg0(0)
            mm_start(1, None)
            for t in range(T):
                g, s = t // RB, t % RB
                if s == 0:
                    hring = rp.tile([128, RB, 2, BL], F16, name="hr",
                                    tag="hring")
                    if (g + 1) * RB < T:
                        get_xr(g + 1)   # prefetch next group's DMA early
                c_ap, u_ap = elem_a(pg_t[t], c_ap)
                if t + 1 < T:
                    mm_finish(t + 1, u_ap)
                # h(t) = P*h(t-1) + u(t)  (off critical path)
                hr_out = hring[:, s, :, :]
                nc.vector.scalar_tensor_tensor(
                    hr_out, h_ap, P_ZO, u_ap, OP.mult, OP.add)
                h_ap = hr_out
                pg_t.pop(t)
                if t + 2 < T:
                    mm_start(t + 2, h_ap)
                pace(PACE)
                if s == RB - 1:
                    nc.sync.dma_start(
                        out_d[g],
                        hring[:].rearrange("p t kc b -> p (t kc b)"))
            while work_q:
                run_one()

    nc.compile()
    return nc


class _LateTile:
    """AP-slicing proxy: resolves tiles[idx] at item-run time (the tile is
    allocated by an earlier queued item)."""

    def __init__(self, tiles, idx):
        self.tiles = tiles
        self.idx = idx

    def __getitem__(self, sl):
        return self.tiles[self.idx][sl]


def _prep_core(inputs, core):
    f32 = np.float32
    fwd = core < 4
    tag = "f" if fwd else "b"
    bsl = slice(8 * (core % 4), 8 * (core % 4) + 8)
    # gate order [i, g, f, o]
    perm = np.concatenate([np.arange(0, H), np.arange(2 * H, 3 * H),
                           np.arange(H, 2 * H), np.arange(3 * H, 4 * H)])

    x = np.asarray(inputs["x"], f32)[bsl].transpose(1, 0, 2)   # [Cin, 8, T]
    if not fwd:
        x = x[:, :, ::-1]
    xp = np.zeros((C_IN, BL, TP), f32)
    xp[:, :, 2:2 + T] = x

    d = {"x": xp.astype(np.float16)}

    bn = np.zeros((128, 3, 2, 4), f32)
    for l in range(3):
        cw = np.asarray(inputs[f"cw{l}"], f32)
        if not fwd:
            cw = cw[:, :, ::-1]
        s = np.asarray(inputs[f"bg{l}"], f32) / np.sqrt(
            np.asarray(inputs[f"bv{l}"], f32) + BN_EPS)
        bias = ((np.asarray(inputs[f"cb{l}"], f32)
                 - np.asarray(inputs[f"bm{l}"], f32)) * s
                + np.asarray(inputs[f"bb{l}"], f32))
        bn[:, l, 0, :] = 1.0               # scale folded into weights
        bn[:, l, 1, :] = bias.reshape(4, 128).T
        wt = cw.transpose(1, 2, 0) * s[None, None, :]   # [cin, K, C] * s
        if l == 0:
            d["w0"] = np.ascontiguousarray(wt).astype(np.float16)
        else:
            d[f"w{l}"] = np.ascontiguousarray(
                wt.reshape(4, 128, K, C).transpose(1, 0, 2, 3)
            ).astype(np.float16)
    d["bn"] = bn

    wih = np.asarray(inputs[f"wih_{tag}"], f32)[perm]          # [1024, 512]
    whh = np.asarray(inputs[f"whh_{tag}"], f32)[perm]          # [1024, 256]
    bg = (np.asarray(inputs[f"bih_{tag}"], f32)
          + np.asarray(inputs[f"bhh_{tag}"], f32))[perm]
    # g-gate rows doubled: kernel computes tanh(g) as 2*sigmoid(2g)-1
    wih = wih.copy(); whh = whh.copy(); bg = bg.copy()
    wih[H:2 * H] *= 2.0
    whh[H:2 * H] *= 2.0
    bg[H:2 * H] *= 2.0
    d["wih"] = np.ascontiguousarray(
        wih.T.reshape(4, 128, 4 * H).transpose(1, 0, 2)).astype(np.float16)
    whh_prep = np.ascontiguousarray(
        whh.T.reshape(2, 128, 4 * H).transpose(1, 0, 2)).astype(np.float16)
    d["whh"] = whh_prep
    d["whhp"] = (np.float32(P_ZO) * whh_prep.astype(np.float32)
                 ).astype(np.float16)
    d["bg"] = bg.reshape(1, 4 * H)
    return d


def kernel(**inputs):
    if "nc" not in _CACHE:
        _CACHE["nc"] = _build()
    nc = _CACHE["nc"]
    in_maps = [_prep_core(inputs, c) for c in range(8)]
    res = run_bass_kernel_spmd(nc, in_maps, list(range(8)))
    _CACHE["last"] = res
    out = np.empty((B, T, 2 * H), np.float32)
    for c in range(8):
        bsl = slice(8 * (c % 4), 8 * (c % 4) + 8)
        arr = np.asarray(res.results[c]["out"], np.float32)
        arr = arr.reshape(T // RB, 128, RB, 2, BL)
        h = arr.transpose(4, 0, 2, 3, 1).reshape(BL, T, H)
        if c < 4:
            out[bsl, :, :H] = h
        else:
            out[bsl, :, H:] = h[:, ::-1, :]
    return out



# revision 9
# speedup vs baseline: 1.0088x; 1.0088x over previous
"""Trainium2 Bass kernel for nn_Encoder (Tacotron2-style encoder):
3x(Conv1d K=5 + BatchNorm(eval) + ReLU) -> bidirectional LSTM (H=256/dir)
with zoneout(p=0.1, eval).

Sharding: 8 cores = 2 directions x 4 batch-groups (8 samples each).
The backward direction runs the SAME program on time-reversed input with
tap-flipped conv weights; the host reverses its output back.

Per-core pipeline:
  A small conv prologue covers t<134 and feeds the first 125-step
  x-projection block so the recurrence can start almost immediately.
  The remaining conv work (BN scale folded into the fp16 weights) +
  x-projections are chopped into ~50-250ns work items, queued in
  time-order (125-step chunks), and paced into the engine gaps of the
  LSTM recurrence.

  The recurrence runs as TWO phase-shifted 4-sample chains.  Each
  chain's per-step dependency cycle is
  u -> 16 Whh@u matmuls -> fused sigmoid over all 4 gates (g
  pre-doubled so tanh(g)=2*sig(2g)-1) -> 3 DVE ops -> tanh -> u, using
  zoneout linearity Whh@h(t) = (P*Whh)@h(t-1) + Whh@u(t) to keep the
  P-part and the h/c state updates off the critical path (h on DVE, c
  on Pool).  The program interleaves the chains at half-step
  granularity (B-h1(k), A-h2(k), A-h1(k+1), B-h2(k)) so each strictly
  in-order engine FIFO sees instructions in data-readiness order; the
  FIFO itself then locks B ~half a cycle behind A and a step completes
  every ~L/2.
"""
import os
from collections import deque

import numpy as np

import concourse.bacc as bacc
import concourse.tile as tile
import concourse.mybir as mybir
from concourse.bass_utils import run_bass_kernel_spmd
from concourse.masks import make_identity

F32 = mybir.dt.float32
F32R = mybir.dt.float32r
F16 = mybir.dt.float16
AF = mybir.ActivationFunctionType
OP = mybir.AluOpType

B, C_IN, T = 32, 80, 1000
C, H, K = 512, 256, 5
BL = 8                       # samples per core
BC = 4                       # samples per chain (2 chains per core)
TP = T + 4                   # padded time
P_ZO = 0.1                   # zoneout keep prob
Q_ZO = 1.0 - P_ZO
BN_EPS = 1e-5
RB = 25                      # steps per ring/out group
NJJ = 8                      # xproj 125-step blocks
CW = 136                     # conv chunk tile width (133 used)

_CACHE = {}


def _build():
    nc = bacc.Bacc("TRN2", target_bir_lowering=False, debug=False,
                   num_devices=8)

    x_d = nc.dram_tensor("x", [C_IN, BL, TP], F16, kind="ExternalInput")
    w0_d = nc.dram_tensor("w0", [C_IN, K, C], F16, kind="ExternalInput")
    w1_d = nc.dram_tensor("w1", [128, 4, K, C], F16, kind="ExternalInput")
    w2_d = nc.dram_tensor("w2", [128, 4, K, C], F16, kind="ExternalInput")
    bn_d = nc.dram_tensor("bn", [128, 3, 2, 4], F32, kind="ExternalInput")
    wih_d = nc.dram_tensor("wih", [128, 4, 4 * H], F16, kind="ExternalInput")
    bg_d = nc.dram_tensor("bg", [1, 4 * H], F32, kind="ExternalInput")
    whh_d = nc.dram_tensor("whh", [128, 2, 4 * H], F16, kind="ExternalInput")
    whhp_d = nc.dram_tensor("whhp", [128, 2, 4 * H], F16,
                            kind="ExternalInput")
    out_d = nc.dram_tensor("out", [T // RB, 128, RB * 2 * BL], F16,
                           kind="ExternalOutput")

    with tile.TileContext(nc) as tc:
        with (
            tc.tile_pool(name="const", bufs=1) as cpool,
            tc.tile_pool(name="blk", bufs=3) as blk,
            tc.tile_pool(name="cps", bufs=2, space="PSUM") as cps,
            tc.tile_pool(name="xsb", bufs=1) as xsb,
            tc.tile_pool(name="gpsA", bufs=3, space="PSUM") as gpsA,
            tc.tile_pool(name="gpsB", bufs=3, space="PSUM") as gpsB,
            tc.tile_pool(name="step", bufs=3) as sp,
            tc.tile_pool(name="ring", bufs=3) as rp,
            tc.tile_pool(name="dram", bufs=1, space="DRAM") as dp,
        ):
            # per-125-step xproj staging buffers in HBM, layout [t,m,p,b]
            xpt = [dp.tile([125, 8, 128, BL], F16, name=f"xp{j}")
                   for j in range(NJJ)]

            # ---- constants / weights in SBUF ----
            x_sb = cpool.tile([C_IN, BL, TP], F16)
            nc.sync.dma_start(x_sb[:], x_d[:])
            w0 = cpool.tile([C_IN, K, C], F16)
            nc.sync.dma_start(w0[:], w0_d[:])
            w1 = cpool.tile([128, 4, K, C], F16, tag="bigw0")
            nc.sync.dma_start(w1[:], w1_d[:])
            w2 = cpool.tile([128, 4, K, C], F16, tag="bigw1")
            nc.sync.dma_start(w2[:], w2_d[:])
            bn = cpool.tile([128, 3, 2, 4], F32)
            nc.sync.dma_start(bn[:], bn_d[:])
            wih = cpool.tile([128, 4, 4 * H], F16)
            nc.sync.dma_start(wih[:], wih_d[:])
            whh = cpool.tile([128, 2, 4 * H], F16)
            nc.sync.dma_start(whh[:], whh_d[:])
            whhp = cpool.tile([128, 2, 4 * H], F16, tag="whhp")
            nc.sync.dma_start(whhp[:], whhp_d[:])
            bgate_f = sp.tile([1, 4 * H], F32, name="bgf", tag="bgf")
            nc.sync.dma_start(bgate_f[:], bg_d[:])
            bgate = cpool.tile([1, 4 * H], F32R)
            nc.vector.tensor_copy(bgate[:], bgate_f[:])
            ones_f = sp.tile([1, 128], F32, name="onesf", tag="onesf")
            nc.gpsimd.memset(ones_f[:], 1.0)
            ones = cpool.tile([1, 128], F32R)
            nc.vector.tensor_copy(ones[:], ones_f[:])
            hzero = cpool.tile([128, 2, BL], F16)
            nc.gpsimd.memset(hzero[:], 0.0)
            czero = cpool.tile([128, 2, BL], F32)
            nc.gpsimd.memset(czero[:], 0.0)
            pconst = cpool.tile([128, 2, BL], F32, tag="pconst")
            nc.gpsimd.memset(pconst[:], P_ZO)
            ident = cpool.tile([128, 128], F16)
            make_identity(nc, ident[:])

            # =========== conv prologue (eager): t < 129-2l ===========
            # quarter tile col c <-> t = c - 6; feeds ONLY xproj block 0
            # (l2 t<125) -- queued chunks recompute their own halos.
            prevq = None
            for l in range(3):
                otq = blk.tile([128, 4, BL, 144], F16, name=f"q{l}",
                               tag="blkq", bufs=2)
                nc.gpsimd.memset(otq[:, :, :, 0:6], 0.0)
                n = 129 - 2 * l
                nm = 4 if l > 0 else 1
                w_l = (w0, w1, w2)[l]
                for m in range(4):
                    for b in range(BL):
                        ps = cps.tile([128, 506], F32, name="cps", tag="cps")
                        first = True
                        for q in range(nm):
                            for k in range(K):
                                if l == 0:
                                    lhsT = w_l[:, k, 128 * m:128 * (m + 1)]
                                    rhs = x_sb[:, b, k:k + n]
                                else:
                                    lhsT = w_l[:, q, k, 128 * m:128 * (m + 1)]
                                    rhs = prevq[:, q, b, 4 + k:4 + k + n]
                                nc.tensor.matmul(ps[:, 0:n], lhsT, rhs,
                                                 start=first,
                                                 stop=(q == nm - 1 and
                                                       k == K - 1))
                                first = False
                        # BN scale folded into weights; alternate the
                        # bias+ReLU epilogue across ACT/DVE to overlap
                        if b % 2 == 0:
                            nc.scalar.activation(
                                otq[:, m, b, 6:6 + n], ps[:, 0:n],
                                AF.Relu, bias=bn[:, l, 1, m:m + 1],
                                scale=bn[:, l, 0, m:m + 1])
                        else:
                            nc.vector.tensor_scalar(
                                otq[:, m, b, 6:6 + n], ps[:, 0:n],
                                bn[:, l, 1, m:m + 1], 0.0,
                                OP.add, OP.max)
                prevq = otq

            def xproj_emit(j, feat, c0, copy_eng):
                """x-projections for steps 125j..125j+124 from feat tile
                (cols c0..c0+125), staged to xpt[j]."""
                stg = xsb.tile([125, 8, 128, BL], F16, name="stg", tag="stg")
                for b in range(BL):
                    for nn2 in range(2):
                        # shares the conv PSUM slots (tag "cps") to free
                        # banks for the two per-chain gate pools
                        ps = cps.tile([125, 512], F32, name="xps", tag="cps")
                        for q in range(4):
                            yield 220, lambda b=b, nn2=nn2, ps=ps, q=q: \
                                nc.tensor.matmul(
                                    ps[:],
                                    feat[:, q, b, c0:c0 + 125],
                                    wih[:, q, 512 * nn2:512 * (nn2 + 1)],
                                    start=(q == 0), stop=False)
                        yield 220, lambda b=b, nn2=nn2, ps=ps: \
                            nc.tensor.matmul(
                                ps[:], ones[:, 0:125],
                                bgate[:, 512 * nn2:512 * (nn2 + 1)],
                                start=False, stop=True)
                        for hf in range(2):
                            yield 460, lambda b=b, nn2=nn2, ps=ps, hf=hf: \
                                copy_eng(
                                    stg[:, 4 * nn2 + 2 * hf:
                                        4 * nn2 + 2 * (hf + 1), :, b],
                                    ps[:, 256 * hf:256 * (hf + 1)]
                                    .rearrange("t (m p) -> t m p", p=128))
                yield 600, lambda: nc.sync.dma_start(xpt[j][:], stg[:])

            # prologue xproj block 0 (eager, copies alternate ACT/DVE)
            _pcnt = [0]

            def _pro_copy(o, i):
                _pcnt[0] += 1
                if _pcnt[0] % 2:
                    nc.scalar.activation(o, i, AF.Copy)
                else:
                    nc.vector.tensor_copy(o, i)

            for cost, fn in xproj_emit(0, prevq, 6, _pro_copy):
                fn()

            # =========== queued conv chunks j=1..7 + xproj ===========
            # chunk j: l0 t in [125j-4, 125j+129), l1 [125j-2, 125j+127),
            # l2 [125j, 125j+125); tile col c <-> t = (125j-4) + c.
            # t >= 1000 halo cols are memset to 0 (zero padding).
            work_q = deque()
            marks = {0: True}
            total_cost = [0.0]

            def push(cost, fn):
                work_q.append((cost, fn))
                total_cost[0] += cost

            def run_one():
                cost, fn = work_q.popleft()
                fn()
                total_cost[0] -= cost
                return cost

            def pace(budget):
                while work_q and budget > 0.0:
                    budget -= run_one()

            def drain_mark(j):
                while not marks.get(j):
                    if not work_q:
                        raise RuntimeError(f"mark {j} never queued")
                    run_one()

            def conv_chunk_items(j, tiles):
                base = 125 * j - 4
                for l in range(3):
                    t_lo = base + 2 * l
                    n = min(t_lo + 133 - 4 * l, 1000) - t_lo
                    c_lo = t_lo - base
                    nm = 4 if l > 0 else 1
                    w_l = (w0, w1, w2)[l]

                    def mk_tile(l=l, t_lo=t_lo, n=n, c_lo=c_lo):
                        ot = blk.tile([128, 4, BL, CW], F16, name=f"ck{l}",
                                      tag="blk")
                        if t_lo + n >= 1000 and c_lo + n < CW:
                            nc.gpsimd.memset(ot[:, :, :, c_lo + n:CW], 0.0)
                        tiles[l] = ot
                    yield 50, mk_tile
                    for m in range(4):
                        for b in range(BL):
                            cell = {}

                            def mk_ps(cell=cell, n=n):
                                cell["ps"] = cps.tile([128, 506], F32,
                                                      name="cps", tag="cps")
                            yield 10, mk_ps
                            for q in range(nm):
                                for k in range(K):
                                    last = (q == nm - 1 and k == K - 1)

                                    def mm(l=l, m=m, b=b, q=q, k=k,
                                           cell=cell, n=n, c_lo=c_lo,
                                           t_lo=t_lo, w_l=w_l, last=last,
                                           first=(q == 0 and k == 0)):
                                        ps = cell["ps"]
                                        if l == 0:
                                            lhsT = w_l[:, k,
                                                       128 * m:128 * (m + 1)]
                                            rhs = x_sb[:, b,
                                                       t_lo + k:t_lo + k + n]
                                        else:
                                            lhsT = w_l[:, q, k,
                                                       128 * m:128 * (m + 1)]
                                            rhs = tiles[l - 1][
                                                :, q, b,
                                                c_lo - 2 + k:c_lo - 2 + k + n]
                                        nc.tensor.matmul(ps[:, 0:n], lhsT,
                                                         rhs, start=first,
                                                         stop=last)
                                    yield n * 0.42 + 3, mm
                            def ep_dve(l=l, m=m, b=b, cell=cell,
                                       c_lo=c_lo, n=n):
                                nc.vector.tensor_scalar(
                                    tiles[l][:, m, b, c_lo:c_lo + n],
                                    cell["ps"][:, 0:n],
                                    bn[:, l, 1, m:m + 1], 0.0,
                                    OP.add, OP.max)
                            yield 280, ep_dve

            # Pool cannot read PSUM -> stage-copies alternate DVE/ACT to
            # split the ~390ns-per-copy load across both engines.
            _qcnt = [0]

            def _q_copy(o, i):
                _qcnt[0] += 1
                if _qcnt[0] % 2:
                    nc.vector.tensor_copy(o, i)
                else:
                    nc.scalar.activation(o, i, AF.Copy)

            for j in range(1, NJJ):
                tiles_j = {}
                for cost, fn in conv_chunk_items(j, tiles_j):
                    push(cost, fn)
                for cost, fn in xproj_emit(
                        j, _LateTile(tiles_j, 2), 4, _q_copy):
                    push(cost, fn)
                push(1, lambda j=j: marks.__setitem__(j, True))

            # ====== recurrence: 2 phase-shifted chains, split matmul ======
            # Zoneout linearity: h(t) = P*h(t-1) + u(t), so
            #   Whh@h(t) = (P*Whh)@h(t-1) + Whh@u(t).
            # The P-part (whhp) runs early, off the critical path; only the
            # 16 Whh@u matmuls sit between u(t) and sigma(t+1).
            xr_tiles = {}

            def get_xr(g):
                if g not in xr_tiles:
                    blkj = (g * RB) // 125
                    drain_mark(blkj)
                    xr = rp.tile([128, RB, 8, BL], F16, name="xr", tag="xr",
                                 bufs=3)
                    toff = g * RB - 125 * blkj
                    nc.sync.dma_start(
                        xr[:],
                        xpt[blkj][toff:toff + RB]
                        .rearrange("t m p b -> p t m b"))
                    xr_tiles[g] = xr
                return xr_tiles[g]

            pg_t = [{}, {}]            # per-chain PSUM gate tiles
            gpools = (gpsA, gpsB)

            def mm_start(ch, t, h_ap):
                """open chain ch's pg(t): xr inject + (P*Whh)@h(t-2)-part
                (h_ap)."""
                if t >= T:
                    return
                g, s = t // RB, t % RB
                xr = get_xr(g)
                pg = gpools[ch].tile([128, 8, BC], F32, name="pg",
                                     tag="gps")
                pg_t[ch][t] = pg
                nc.tensor.matmul(pg[:], ident,
                                 xr[:, s, :, BC * ch:BC * (ch + 1)],
                                 start=True, stop=False)
                if h_ap is not None:
                    for m in range(8):
                        for kc in range(2):
                            nc.tensor.matmul(
                                pg[:, m, :],
                                whhp[:, kc, 128 * m:128 * (m + 1)],
                                h_ap[:, kc, :],
                                start=False, stop=False)

            def mm_finish(ch, t, u_ap):
                """close chain ch's pg(t): Whh@u(t-1)-part."""
                pg = pg_t[ch][t]
                for m in range(8):
                    for kc in range(2):
                        nc.tensor.matmul(
                            pg[:, m, :],
                            whh[:, kc, 128 * m:128 * (m + 1)],
                            u_ap[:, kc, :],
                            start=False, stop=(m == 7 and kc == 1))

            # elementwise step; m-blocks 0:2=i, 2:4=g(2x), 4:6=f, 6:8=o.
            #   tanh(g) = 2*sig(2g)-1:
            #   wv = (sig2g - 0.5)*sigi;  v2 = Q*sigf*c
            #   w  = 2Q*wv + v2 = Q*c2;   c' = P*c + w
            #   tc = tanh(w/Q);  u = Q*sigo*tc;  h' = P*h + u
            # Split into two halves so the two chains interleave at
            # half-step granularity (engine FIFOs are strictly in-order:
            # program order must match data-readiness order).
            def elem_h1(ch, t, c_ap):
                sx = str(ch)
                pg = pg_t[ch].pop(t)
                sall = sp.tile([128, 8, BC], F16, name="sall",
                               tag="sall" + sx, bufs=5)
                nc.scalar.activation(sall[:], pg[:], AF.Sigmoid)
                wv = sp.tile([128, 2, BC], F16, name="wv", tag="wv" + sx,
                             bufs=5)
                nc.vector.scalar_tensor_tensor(
                    wv[:], sall[:, 2:4, :], 0.5, sall[:, 0:2, :],
                    OP.subtract, OP.mult)
                v2 = sp.tile([128, 2, BC], F16, name="v2", tag="v2" + sx,
                             bufs=5)
                nc.vector.scalar_tensor_tensor(
                    v2[:], sall[:, 4:6, :], Q_ZO, c_ap, OP.mult, OP.mult)
                w_t = sp.tile([128, 2, BC], F32, name="w", tag="w" + sx,
                              bufs=5)
                nc.vector.scalar_tensor_tensor(
                    w_t[:], wv[:], 2.0 * Q_ZO, v2[:], OP.mult, OP.add)
                # c' = P*c + w on Pool (2 tensor_tensor ops: Pool rejects
                # TensorScalarPtr), freeing a DVE queue slot
                cp_ = sp.tile([128, 2, BC], F32, name="cp", tag="cp" + sx,
                              bufs=5)
                nc.gpsimd.tensor_tensor(
                    cp_[:], c_ap, pconst[:, :, BC * ch:BC * (ch + 1)],
                    OP.mult)
                c_new = sp.tile([128, 2, BC], F32, name="c", tag="c" + sx,
                                bufs=5)
                nc.gpsimd.tensor_tensor(c_new[:], cp_[:], w_t[:], OP.add)
                return sall, w_t, c_new[:]

            def elem_h2(ch, sall, w_t):
                sx = str(ch)
                tc2 = sp.tile([128, 2, BC], F16, name="tc2", tag="tc2" + sx,
                              bufs=5)
                nc.scalar.activation(tc2[:], w_t[:], AF.Tanh,
                                     scale=1.0 / Q_ZO)
                u = sp.tile([128, 2, BC], F16, name="u", tag="u" + sx,
                            bufs=5)
                nc.vector.scalar_tensor_tensor(
                    u[:], sall[:, 6:8, :], Q_ZO, tc2[:], OP.mult, OP.mult)
                return u[:]

            hring = [None]
            h_ap = [hzero[:, :, 0:BC], hzero[:, :, BC:2 * BC]]
            c_ap = [czero[:, :, 0:BC], czero[:, :, BC:2 * BC]]
            PACE = float(os.environ.get("ENC_PACE", "1150"))

            def close_pg0(ch):
                nc.tensor.matmul(pg_t[ch][0][:, 0, :], ident[:, 0:128],
                                 hzero[:, 0, BC * ch:BC * (ch + 1)],
                                 start=False, stop=True)

            def half2(ch, t, st):
                """tanh/u + recurrent matmuls + h-update for step t."""
                g, s = t // RB, t % RB
                sall, w_t, _ = st
                if ch == 0 and s == 0:
                    hring[0] = rp.tile([128, RB, 2, BL], F16, name="hr",
                                       tag="hring")
                    if (g + 1) * RB < T:
                        get_xr(g + 1)   # prefetch next group's DMA early
                u_new = elem_h2(ch, sall, w_t)
                if t + 1 < T:
                    mm_finish(ch, t + 1, u_new)
                # h(t) = P*h(t-1) + u(t)  (off critical path)
                hr_out = hring[0][:, s, :, BC * ch:BC * (ch + 1)]
                nc.vector.scalar_tensor_tensor(
                    hr_out, h_ap[ch], P_ZO, u_new, OP.mult, OP.add)
                h_ap[ch] = hr_out
                if t + 2 < T:
                    mm_start(ch, t + 2, h_ap[ch])
                if ch == 1 and s == RB - 1:
                    nc.sync.dma_start(
                        out_d[g],
                        hring[0].rearrange("p t kc b -> p (t kc b)"))

            # warmup: open pg(0)/pg(1) for both chains, close both pg(0),
            # then run A's first half so the steady loop can start with B.
            mm_start(0, 0, None)
            close_pg0(0)
            mm_start(1, 0, None)
            close_pg0(1)
            mm_start(0, 1, None)
            mm_start(1, 1, None)
            stA = elem_h1(0, 0, c_ap[0])
            c_ap[0] = stA[2]
            stB = None

            # steady loop; per iteration k the engine-FIFO order is
            #   B-h1(k), A-h2(k), A-h1(k+1), B-h2(k)
            # which matches readiness when B runs ~half a cycle behind A.
            for k in range(T):
                stB = elem_h1(1, k, c_ap[1])
                c_ap[1] = stB[2]
                half2(0, k, stA)
                pace(PACE * 0.5)
                if k + 1 < T:
                    stA = elem_h1(0, k + 1, c_ap[0])
                    c_ap[0] = stA[2]
                half2(1, k, stB)
                pace(PACE * 0.5)
            while work_q:
                run_one()

    nc.compile()
    return nc


class _LateTile:
    """AP-slicing proxy: resolves tiles[idx] at item-run time (the tile is
    allocated by an earlier queued item)."""

    def __init__(self, tiles, idx):
        self.tiles = tiles
        self.idx = idx

    def __getitem__(self, sl):
        return self.tiles[self.idx][sl]


def _prep_core(inputs, core):
    f32 = np.float32
    fwd = core < 4
    tag = "f" if fwd else "b"
    bsl = slice(8 * (core % 4), 8 * (core % 4) + 8)
    # gate order [i, g, f, o]
    perm = np.concatenate([np.arange(0, H), np.arange(2 * H, 3 * H),
                           np.arange(H, 2 * H), np.arange(3 * H, 4 * H)])

    x = np.asarray(inputs["x"], f32)[bsl].transpose(1, 0, 2)   # [Cin, 8, T]
    if not fwd:
        x = x[:, :, ::-1]
    xp = np.zeros((C_IN, BL, TP), f32)
    xp[:, :, 2:2 + T] = x

    d = {"x": xp.astype(np.float16)}

    bn = np.zeros((128, 3, 2, 4), f32)
    for l in range(3):
        cw = np.asarray(inputs[f"cw{l}"], f32)
        if not fwd:
            cw = cw[:, :, ::-1]
        s = np.asarray(inputs[f"bg{l}"], f32) / np.sqrt(
            np.asarray(inputs[f"bv{l}"], f32) + BN_EPS)
        bias = ((np.asarray(inputs[f"cb{l}"], f32)
                 - np.asarray(inputs[f"bm{l}"], f32)) * s
                + np.asarray(inputs[f"bb{l}"], f32))
        bn[:, l, 0, :] = 1.0               # scale folded into weights
        bn[:, l, 1, :] = bias.reshape(4, 128).T
        wt = cw.transpose(1, 2, 0) * s[None, None, :]   # [cin, K, C] * s
        if l == 0:
            d["w0"] = np.ascontiguousarray(wt).astype(np.float16)
        else:
            d[f"w{l}"] = np.ascontiguousarray(
                wt.reshape(4, 128, K, C).transpose(1, 0, 2, 3)
            ).astype(np.float16)
    d["bn"] = bn

    wih = np.asarray(inputs[f"wih_{tag}"], f32)[perm]          # [1024, 512]
    whh = np.asarray(inputs[f"whh_{tag}"], f32)[perm]          # [1024, 256]
    bg = (np.asarray(inputs[f"bih_{tag}"], f32)
          + np.asarray(inputs[f"bhh_{tag}"], f32))[perm]
    # g-gate rows doubled: kernel computes tanh(g) as 2*sigmoid(2g)-1
    wih = wih.copy(); whh = whh.copy(); bg = bg.copy()
    wih[H:2 * H] *= 2.0
    whh[H:2 * H] *= 2.0
    bg[H:2 * H] *= 2.0
    d["wih"] = np.ascontiguousarray(
        wih.T.reshape(4, 128, 4 * H).transpose(1, 0, 2)).astype(np.float16)
    whh_prep = np.ascontiguousarray(
        whh.T.reshape(2, 128, 4 * H).transpose(1, 0, 2)).astype(np.float16)
    d["whh"] = whh_prep
    d["whhp"] = (np.float32(P_ZO) * whh_prep.astype(np.float32)
                 ).astype(np.float16)
    d["bg"] = bg.reshape(1, 4 * H)
    return d


def kernel(**inputs):
    if "nc" not in _CACHE:
        _CACHE["nc"] = _build()
    nc = _CACHE["nc"]
    in_maps = [_prep_core(inputs, c) for c in range(8)]
    res = run_bass_kernel_spmd(nc, in_maps, list(range(8)))
    _CACHE["last"] = res
    out = np.empty((B, T, 2 * H), np.float32)
    for c in range(8):
        bsl = slice(8 * (c % 4), 8 * (c % 4) + 8)
        arr = np.asarray(res.results[c]["out"], np.float32)
        arr = arr.reshape(T // RB, 128, RB, 2, BL)
        h = arr.transpose(4, 0, 2, 3, 1).reshape(BL, T, H)
        if c < 4:
            out[bsl, :, :H] = h
        else:
            out[bsl, :, H:] = h[:, ::-1, :]
    return out


# revision 13
# speedup vs baseline: 1.0105x; 1.0016x over previous
"""Trainium2 Bass kernel for nn_Encoder (Tacotron2-style encoder):
3x(Conv1d K=5 + BatchNorm(eval) + ReLU) -> bidirectional LSTM (H=256/dir)
with zoneout(p=0.1, eval).

Sharding: 8 cores = 2 directions x 4 batch-groups (8 samples each).
The backward direction runs the SAME program on time-reversed input with
tap-flipped conv weights; the host reverses its output back.

Per-core pipeline:
  A small conv prologue covers t<134 and feeds the first 125-step
  x-projection block so the recurrence can start almost immediately.
  The remaining conv work (BN scale folded into the fp16 weights) +
  x-projections are chopped into ~50-250ns work items, queued in
  time-order (125-step chunks), and paced into the engine gaps of the
  LSTM recurrence.

  The recurrence runs as TWO phase-shifted 4-sample chains.  Each
  chain's per-step dependency cycle is
  u -> 16 Whh@u matmuls -> fused sigmoid over all 4 gates (g
  pre-doubled so tanh(g)=2*sig(2g)-1) -> 3 DVE ops -> tanh -> u, using
  zoneout linearity Whh@h(t) = (P*Whh)@h(t-1) + Whh@u(t) to keep the
  P-part and the h/c state updates off the critical path (h on DVE, c
  on Pool).  The program interleaves the chains at half-step
  granularity (B-h1(k), A-h2(k), A-h1(k+1), B-h2(k)) so each strictly
  in-order engine FIFO sees instructions in data-readiness order; the
  FIFO itself then locks B ~half a cycle behind A and a step completes
  every ~L/2.
"""
import os
from collections import deque

import numpy as np

import concourse.bacc as bacc
import concourse.tile as tile
import concourse.mybir as mybir
from concourse.bass_utils import run_bass_kernel_spmd
from concourse.masks import make_identity

F32 = mybir.dt.float32
F32R = mybir.dt.float32r
F16 = mybir.dt.float16
AF = mybir.ActivationFunctionType
OP = mybir.AluOpType

B, C_IN, T = 32, 80, 1000
C, H, K = 512, 256, 5
BL = 8                       # samples per core
BC = 4                       # samples per chain (2 chains per core)
TP = T + 4                   # padded time
P_ZO = 0.1                   # zoneout keep prob
Q_ZO = 1.0 - P_ZO
BN_EPS = 1e-5
RB = 25                      # steps per ring/out group
NJJ = 8                      # xproj 125-step blocks
CW = 136                     # conv chunk tile width (133 used)

_CACHE = {}


def _build():
    nc = bacc.Bacc("TRN2", target_bir_lowering=False, debug=False,
                   num_devices=8)

    x_d = nc.dram_tensor("x", [C_IN, BL, TP], F16, kind="ExternalInput")
    w0_d = nc.dram_tensor("w0", [C_IN, K, C], F16, kind="ExternalInput")
    w1_d = nc.dram_tensor("w1", [128, 4, K, C], F16, kind="ExternalInput")
    w2_d = nc.dram_tensor("w2", [128, 4, K, C], F16, kind="ExternalInput")
    bn_d = nc.dram_tensor("bn", [128, 3, 2, 4], F32, kind="ExternalInput")
    wih_d = nc.dram_tensor("wih", [128, 4, 4 * H], F16, kind="ExternalInput")
    bg_d = nc.dram_tensor("bg", [1, 4 * H], F32, kind="ExternalInput")
    whh_d = nc.dram_tensor("whh", [128, 2, 4 * H], F16, kind="ExternalInput")
    whhp_d = nc.dram_tensor("whhp", [128, 2, 4 * H], F16,
                            kind="ExternalInput")
    out_d = nc.dram_tensor("out", [T // RB, 128, RB * 2 * BL], F16,
                           kind="ExternalOutput")

    with tile.TileContext(nc) as tc:
        with (
            tc.tile_pool(name="const", bufs=1) as cpool,
            tc.tile_pool(name="blk", bufs=3) as blk,
            tc.tile_pool(name="cps", bufs=2, space="PSUM") as cps,
            tc.tile_pool(name="xsb", bufs=1) as xsb,
            tc.tile_pool(name="gpsA", bufs=3, space="PSUM") as gpsA,
            tc.tile_pool(name="gpsB", bufs=3, space="PSUM") as gpsB,
            tc.tile_pool(name="step", bufs=3) as sp,
            tc.tile_pool(name="ring", bufs=3) as rp,
            tc.tile_pool(name="dram", bufs=1, space="DRAM") as dp,
        ):
            # per-125-step xproj staging buffers in HBM, layout [t,m,p,b]
            xpt = [dp.tile([125, 8, 128, BL], F16, name=f"xp{j}")
                   for j in range(NJJ)]

            # ---- constants / weights in SBUF ----
            x_sb = cpool.tile([C_IN, BL, TP], F16)
            nc.sync.dma_start(x_sb[:], x_d[:])
            w0 = cpool.tile([C_IN, K, C], F16)
            nc.sync.dma_start(w0[:], w0_d[:])
            w1 = cpool.tile([128, 4, K, C], F16, tag="bigw0")
            nc.sync.dma_start(w1[:], w1_d[:])
            w2 = cpool.tile([128, 4, K, C], F16, tag="bigw1")
            nc.sync.dma_start(w2[:], w2_d[:])
            bn = cpool.tile([128, 3, 2, 4], F32)
            nc.sync.dma_start(bn[:], bn_d[:])
            wih = cpool.tile([128, 4, 4 * H], F16)
            nc.sync.dma_start(wih[:], wih_d[:])
            whh = cpool.tile([128, 2, 4 * H], F16)
            nc.sync.dma_start(whh[:], whh_d[:])
            whhp = cpool.tile([128, 2, 4 * H], F16, tag="whhp")
            nc.sync.dma_start(whhp[:], whhp_d[:])
            bgate_f = sp.tile([1, 4 * H], F32, name="bgf", tag="bgf")
            nc.sync.dma_start(bgate_f[:], bg_d[:])
            bgate = cpool.tile([1, 4 * H], F32R)
            nc.vector.tensor_copy(bgate[:], bgate_f[:])
            ones_f = sp.tile([1, 128], F32, name="onesf", tag="onesf")
            nc.gpsimd.memset(ones_f[:], 1.0)
            ones = cpool.tile([1, 128], F32R)
            nc.vector.tensor_copy(ones[:], ones_f[:])
            hzero = cpool.tile([128, 2, BL], F16)
            nc.gpsimd.memset(hzero[:], 0.0)
            czero = cpool.tile([128, 2, BL], F32)
            nc.gpsimd.memset(czero[:], 0.0)
            pconst = cpool.tile([128, 2, BL], F32, tag="pconst")
            nc.gpsimd.memset(pconst[:], P_ZO)
            ident = cpool.tile([128, 128], F16)
            make_identity(nc, ident[:])

            # =========== conv prologue (eager): t < 129-2l ===========
            # quarter tile col c <-> t = c - 6; feeds ONLY xproj block 0
            # (l2 t<125) -- queued chunks recompute their own halos.
            prevq = None
            for l in range(3):
                otq = blk.tile([128, 4, BL, 144], F16, name=f"q{l}",
                               tag="blkq", bufs=2)
                nc.gpsimd.memset(otq[:, :, :, 0:6], 0.0)
                n = 129 - 2 * l
                nm = 4 if l > 0 else 1
                w_l = (w0, w1, w2)[l]
                for m in range(4):
                    for b in range(BL):
                        ps = cps.tile([128, 506], F32, name="cps", tag="cps")
                        first = True
                        for q in range(nm):
                            for k in range(K):
                                if l == 0:
                                    lhsT = w_l[:, k, 128 * m:128 * (m + 1)]
                                    rhs = x_sb[:, b, k:k + n]
                                else:
                                    lhsT = w_l[:, q, k, 128 * m:128 * (m + 1)]
                                    rhs = prevq[:, q, b, 4 + k:4 + k + n]
                                nc.tensor.matmul(ps[:, 0:n], lhsT, rhs,
                                                 start=first,
                                                 stop=(q == nm - 1 and
                                                       k == K - 1))
                                first = False
                        # BN scale folded into weights; alternate the
                        # bias+ReLU epilogue across ACT/DVE to overlap
                        if b % 2 == 0:
                            nc.scalar.activation(
                                otq[:, m, b, 6:6 + n], ps[:, 0:n],
                                AF.Relu, bias=bn[:, l, 1, m:m + 1],
                                scale=bn[:, l, 0, m:m + 1])
                        else:
                            nc.vector.tensor_scalar(
                                otq[:, m, b, 6:6 + n], ps[:, 0:n],
                                bn[:, l, 1, m:m + 1], 0.0,
                                OP.add, OP.max)
                prevq = otq

            def xproj_emit(j, feat, c0, copy_eng):
                """x-projections for steps 125j..125j+124 from feat tile
                (cols c0..c0+125), staged to xpt[j]."""
                stg = xsb.tile([125, 8, 128, BL], F16, name="stg", tag="stg")
                for b in range(BL):
                    for nn2 in range(2):
                        # shares the conv PSUM slots (tag "cps") to free
                        # banks for the two per-chain gate pools
                        ps = cps.tile([125, 512], F32, name="xps", tag="cps")
                        for q in range(4):
                            yield 220, lambda b=b, nn2=nn2, ps=ps, q=q: \
                                nc.tensor.matmul(
                                    ps[:],
                                    feat[:, q, b, c0:c0 + 125],
                                    wih[:, q, 512 * nn2:512 * (nn2 + 1)],
                                    start=(q == 0), stop=False)
                        yield 220, lambda b=b, nn2=nn2, ps=ps: \
                            nc.tensor.matmul(
                                ps[:], ones[:, 0:125],
                                bgate[:, 512 * nn2:512 * (nn2 + 1)],
                                start=False, stop=True)
                        for hf in range(4):
                            yield 270, lambda b=b, nn2=nn2, ps=ps, hf=hf: \
                                copy_eng(
                                    stg[:, 4 * nn2 + hf:
                                        4 * nn2 + hf + 1, :, b],
                                    ps[:, 128 * hf:128 * (hf + 1)]
                                    .rearrange("t (m p) -> t m p", p=128))
                yield 600, lambda: nc.sync.dma_start(xpt[j][:], stg[:])

            # prologue xproj block 0 (eager, copies alternate ACT/DVE)
            _pcnt = [0]

            def _pro_copy(o, i):
                _pcnt[0] += 1
                if _pcnt[0] % 2:
                    nc.scalar.activation(o, i, AF.Copy)
                else:
                    nc.vector.tensor_copy(o, i)

            for cost, fn in xproj_emit(0, prevq, 6, _pro_copy):
                fn()

            # =========== queued conv chunks j=1..7 + xproj ===========
            # chunk j: l0 t in [125j-4, 125j+129), l1 [125j-2, 125j+127),
            # l2 [125j, 125j+125); tile col c <-> t = (125j-4) + c.
            # t >= 1000 halo cols are memset to 0 (zero padding).
            work_q = deque()
            marks = {0: True}
            total_cost = [0.0]

            def push(cost, fn):
                work_q.append((cost, fn))
                total_cost[0] += cost

            def run_one():
                cost, fn = work_q.popleft()
                fn()
                total_cost[0] -= cost
                return cost

            def pace(budget):
                while work_q and budget > 0.0:
                    budget -= run_one()

            def drain_mark(j):
                while not marks.get(j):
                    if not work_q:
                        raise RuntimeError(f"mark {j} never queued")
                    run_one()

            def conv_chunk_items(j, tiles):
                base = 125 * j - 4
                for l in range(3):
                    t_lo = base + 2 * l
                    n = min(t_lo + 133 - 4 * l, 1000) - t_lo
                    c_lo = t_lo - base
                    nm = 4 if l > 0 else 1
                    w_l = (w0, w1, w2)[l]

                    def mk_tile(l=l, t_lo=t_lo, n=n, c_lo=c_lo):
                        ot = blk.tile([128, 4, BL, CW], F16, name=f"ck{l}",
                                      tag="blk")
                        if t_lo + n >= 1000 and c_lo + n < CW:
                            nc.gpsimd.memset(ot[:, :, :, c_lo + n:CW], 0.0)
                        tiles[l] = ot
                    yield 50, mk_tile
                    for m in range(4):
                        for b in range(BL):
                            cell = {}

                            def mk_ps(cell=cell, n=n):
                                cell["ps"] = cps.tile([128, 506], F32,
                                                      name="cps", tag="cps")
                            yield 10, mk_ps
                            for q in range(nm):
                                for k in range(K):
                                    last = (q == nm - 1 and k == K - 1)

                                    def mm(l=l, m=m, b=b, q=q, k=k,
                                           cell=cell, n=n, c_lo=c_lo,
                                           t_lo=t_lo, w_l=w_l, last=last,
                                           first=(q == 0 and k == 0)):
                                        ps = cell["ps"]
                                        if l == 0:
                                            lhsT = w_l[:, k,
                                                       128 * m:128 * (m + 1)]
                                            rhs = x_sb[:, b,
                                                       t_lo + k:t_lo + k + n]
                                        else:
                                            lhsT = w_l[:, q, k,
                                                       128 * m:128 * (m + 1)]
                                            rhs = tiles[l - 1][
                                                :, q, b,
                                                c_lo - 2 + k:c_lo - 2 + k + n]
                                        nc.tensor.matmul(ps[:, 0:n], lhsT,
                                                         rhs, start=first,
                                                         stop=last)
                                    yield n * 0.42 + 3, mm
                            def ep_dve(l=l, m=m, b=b, cell=cell,
                                       c_lo=c_lo, n=n):
                                nc.vector.tensor_scalar(
                                    tiles[l][:, m, b, c_lo:c_lo + n],
                                    cell["ps"][:, 0:n],
                                    bn[:, l, 1, m:m + 1], 0.0,
                                    OP.add, OP.max)
                            yield 280, ep_dve

            # Pool cannot read PSUM; keep stage-copies off ACT entirely so
            # the 4-deep ACT wait queue holds only the recurrence sig/tanh.
            def _q_copy(o, i):
                nc.vector.tensor_copy(o, i)

            for j in range(1, NJJ):
                tiles_j = {}
                for cost, fn in conv_chunk_items(j, tiles_j):
                    push(cost, fn)
                for cost, fn in xproj_emit(
                        j, _LateTile(tiles_j, 2), 4, _q_copy):
                    push(cost, fn)
                push(1, lambda j=j: marks.__setitem__(j, True))

            # ====== recurrence: 2 phase-shifted chains, split matmul ======
            # Zoneout linearity: h(t) = P*h(t-1) + u(t), so
            #   Whh@h(t) = (P*Whh)@h(t-1) + Whh@u(t).
            # The P-part (whhp) runs early, off the critical path; only the
            # 16 Whh@u matmuls sit between u(t) and sigma(t+1).
            xr_tiles = {}

            def get_xr(g):
                if g not in xr_tiles:
                    blkj = (g * RB) // 125
                    drain_mark(blkj)
                    xr = rp.tile([128, RB, 8, BL], F16, name="xr", tag="xr",
                                 bufs=3)
                    toff = g * RB - 125 * blkj
                    nc.sync.dma_start(
                        xr[:],
                        xpt[blkj][toff:toff + RB]
                        .rearrange("t m p b -> p t m b"))
                    xr_tiles[g] = xr
                return xr_tiles[g]

            pg_t = [{}, {}]            # per-chain PSUM gate tiles
            gpools = (gpsA, gpsB)

            def mm_start(ch, t, h_ap):
                """open chain ch's pg(t): xr inject + (P*Whh)@h(t-2)-part
                (h_ap)."""
                if t >= T:
                    return
                g, s = t // RB, t % RB
                xr = get_xr(g)
                pg = gpools[ch].tile([128, 8, BC], F32, name="pg",
                                     tag="gps")
                pg_t[ch][t] = pg
                nc.tensor.matmul(pg[:], ident,
                                 xr[:, s, :, BC * ch:BC * (ch + 1)],
                                 start=True, stop=False)
                if h_ap is not None:
                    for m in range(8):
                        for kc in range(2):
                            nc.tensor.matmul(
                                pg[:, m, :],
                                whhp[:, kc, 128 * m:128 * (m + 1)],
                                h_ap[:, kc, :],
                                start=False, stop=False)

            def mm_finish(ch, t, u_ap):
                """close chain ch's pg(t): Whh@u(t-1)-part."""
                pg = pg_t[ch][t]
                for m in range(8):
                    for kc in range(2):
                        nc.tensor.matmul(
                            pg[:, m, :],
                            whh[:, kc, 128 * m:128 * (m + 1)],
                            u_ap[:, kc, :],
                            start=False, stop=(m == 7 and kc == 1))

            # elementwise step; m-blocks 0:2=i, 2:4=g(2x), 4:6=f, 6:8=o.
            #   tanh(g) = 2*sig(2g)-1:
            #   wv = (sig2g - 0.5)*sigi;  v2 = Q*sigf*c
            #   w  = 2Q*wv + v2 = Q*c2;   c' = P*c + w
            #   tc = tanh(w/Q);  u = Q*sigo*tc;  h' = P*h + u
            # Split into two halves so the two chains interleave at
            # half-step granularity (engine FIFOs are strictly in-order:
            # program order must match data-readiness order).
            def elem_h1(ch, t, c_ap):
                sx = str(ch)
                pg = pg_t[ch].pop(t)
                sall = sp.tile([128, 8, BC], F16, name="sall",
                               tag="sall" + sx, bufs=5)
                nc.scalar.activation(sall[:], pg[:], AF.Sigmoid)
                wv = sp.tile([128, 2, BC], F16, name="wv", tag="wv" + sx,
                             bufs=5)
                nc.vector.scalar_tensor_tensor(
                    wv[:], sall[:, 2:4, :], 0.5, sall[:, 0:2, :],
                    OP.subtract, OP.mult)
                v2 = sp.tile([128, 2, BC], F16, name="v2", tag="v2" + sx,
                             bufs=5)
                nc.vector.scalar_tensor_tensor(
                    v2[:], sall[:, 4:6, :], Q_ZO, c_ap, OP.mult, OP.mult)
                w_t = sp.tile([128, 2, BC], F32, name="w", tag="w" + sx,
                              bufs=5)
                nc.vector.scalar_tensor_tensor(
                    w_t[:], wv[:], 2.0 * Q_ZO, v2[:], OP.mult, OP.add)
                # c' = P*c + w on Pool (2 tensor_tensor ops: Pool rejects
                # TensorScalarPtr), freeing a DVE queue slot
                cp_ = sp.tile([128, 2, BC], F32, name="cp", tag="cp" + sx,
                              bufs=5)
                nc.gpsimd.tensor_tensor(
                    cp_[:], c_ap, pconst[:, :, BC * ch:BC * (ch + 1)],
                    OP.mult)
                c_new = sp.tile([128, 2, BC], F32, name="c", tag="c" + sx,
                                bufs=5)
                nc.gpsimd.tensor_tensor(c_new[:], cp_[:], w_t[:], OP.add)
                return sall, w_t, c_new[:]

            def elem_h2(ch, sall, w_t):
                sx = str(ch)
                tc2 = sp.tile([128, 2, BC], F16, name="tc2", tag="tc2" + sx,
                              bufs=5)
                nc.scalar.activation(tc2[:], w_t[:], AF.Tanh,
                                     scale=1.0 / Q_ZO)
                u = sp.tile([128, 2, BC], F16, name="u", tag="u" + sx,
                            bufs=5)
                nc.vector.scalar_tensor_tensor(
                    u[:], sall[:, 6:8, :], Q_ZO, tc2[:], OP.mult, OP.mult)
                return u[:]

            hring = [None]
            h_ap = [hzero[:, :, 0:BC], hzero[:, :, BC:2 * BC]]
            c_ap = [czero[:, :, 0:BC], czero[:, :, BC:2 * BC]]
            PACE = float(os.environ.get("ENC_PACE", "1150"))

            def close_pg0(ch):
                nc.tensor.matmul(pg_t[ch][0][:, 0, :], ident[:, 0:128],
                                 hzero[:, 0, BC * ch:BC * (ch + 1)],
                                 start=False, stop=True)

            def half2(ch, t, st):
                """tanh/u + recurrent matmuls + h-update for step t."""
                g, s = t // RB, t % RB
                sall, w_t, _ = st
                if ch == 0 and s == 0:
                    hring[0] = rp.tile([128, RB, 2, BL], F16, name="hr",
                                       tag="hring")
                    # prefetch 2 groups ahead: one xr DMA costs ~40us of SP
                    # sequencer (25k descriptors) vs ~52us per ring group
                    for ga in (g + 1, g + 2):
                        if ga * RB < T:
                            get_xr(ga)
                u_new = elem_h2(ch, sall, w_t)
                if t + 1 < T:
                    mm_finish(ch, t + 1, u_new)
                # h(t) = P*h(t-1) + u(t)  (off critical path)
                hr_out = hring[0][:, s, :, BC * ch:BC * (ch + 1)]
                nc.vector.scalar_tensor_tensor(
                    hr_out, h_ap[ch], P_ZO, u_new, OP.mult, OP.add)
                h_ap[ch] = hr_out
                if t + 2 < T:
                    mm_start(ch, t + 2, h_ap[ch])
                if ch == 1 and s == RB - 1:
                    nc.sync.dma_start(
                        out_d[g],
                        hring[0].rearrange("p t kc b -> p (t kc b)"))

            # warmup: open pg(0)/pg(1) for both chains, close both pg(0),
            # then run A's first half so the steady loop can start with B.
            mm_start(0, 0, None)
            close_pg0(0)
            mm_start(1, 0, None)
            close_pg0(1)
            mm_start(0, 1, None)
            mm_start(1, 1, None)
            stA = elem_h1(0, 0, c_ap[0])
            c_ap[0] = stA[2]
            stB = None

            # steady loop; per iteration k the engine-FIFO order is
            #   B-h1(k), A-h2(k), A-h1(k+1), B-h2(k)
            # which matches readiness when B runs ~half a cycle behind A.
            for k in range(T):
                stB = elem_h1(1, k, c_ap[1])
                pace(PACE * 0.25)
                c_ap[1] = stB[2]
                half2(0, k, stA)
                pace(PACE * 0.25)
                if k + 1 < T:
                    stA = elem_h1(0, k + 1, c_ap[0])
                    c_ap[0] = stA[2]
                pace(PACE * 0.25)
                half2(1, k, stB)
                pace(PACE * 0.25)
            while work_q:
                run_one()

    nc.compile()
    return nc


class _LateTile:
    """AP-slicing proxy: resolves tiles[idx] at item-run time (the tile is
    allocated by an earlier queued item)."""

    def __init__(self, tiles, idx):
        self.tiles = tiles
        self.idx = idx

    def __getitem__(self, sl):
        return self.tiles[self.idx][sl]


def _prep_core(inputs, core):
    f32 = np.float32
    fwd = core < 4
    tag = "f" if fwd else "b"
    bsl = slice(8 * (core % 4), 8 * (core % 4) + 8)
    # gate order [i, g, f, o]
    perm = np.concatenate([np.arange(0, H), np.arange(2 * H, 3 * H),
                           np.arange(H, 2 * H), np.arange(3 * H, 4 * H)])

    x = np.asarray(inputs["x"], f32)[bsl].transpose(1, 0, 2)   # [Cin, 8, T]
    if not fwd:
        x = x[:, :, ::-1]
    xp = np.zeros((C_IN, BL, TP), f32)
    xp[:, :, 2:2 + T] = x

    d = {"x": xp.astype(np.float16)}

    bn = np.zeros((128, 3, 2, 4), f32)
    for l in range(3):
        cw = np.asarray(inputs[f"cw{l}"], f32)
        if not fwd:
            cw = cw[:, :, ::-1]
        s = np.asarray(inputs[f"bg{l}"], f32) / np.sqrt(
            np.asarray(inputs[f"bv{l}"], f32) + BN_EPS)
        bias = ((np.asarray(inputs[f"cb{l}"], f32)
                 - np.asarray(inputs[f"bm{l}"], f32)) * s
                + np.asarray(inputs[f"bb{l}"], f32))
        bn[:, l, 0, :] = 1.0               # scale folded into weights
        bn[:, l, 1, :] = bias.reshape(4, 128).T
        wt = cw.transpose(1, 2, 0) * s[None, None, :]   # [cin, K, C] * s
        if l == 0:
            d["w0"] = np.ascontiguousarray(wt).astype(np.float16)
        else:
            d[f"w{l}"] = np.ascontiguousarray(
                wt.reshape(4, 128, K, C).transpose(1, 0, 2, 3)
            ).astype(np.float16)
    d["bn"] = bn

    wih = np.asarray(inputs[f"wih_{tag}"], f32)[perm]          # [1024, 512]
    whh = np.asarray(inputs[f"whh_{tag}"], f32)[perm]          # [1024, 256]
    bg = (np.asarray(inputs[f"bih_{tag}"], f32)
          + np.asarray(inputs[f"bhh_{tag}"], f32))[perm]
    # g-gate rows doubled: kernel computes tanh(g) as 2*sigmoid(2g)-1
    wih = wih.copy(); whh = whh.copy(); bg = bg.copy()
    wih[H:2 * H] *= 2.0
    whh[H:2 * H] *= 2.0
    bg[H:2 * H] *= 2.0
    d["wih"] = np.ascontiguousarray(
        wih.T.reshape(4, 128, 4 * H).transpose(1, 0, 2)).astype(np.float16)
    whh_prep = np.ascontiguousarray(
        whh.T.reshape(2, 128, 4 * H).transpose(1, 0, 2)).astype(np.float16)
    d["whh"] = whh_prep
    d["whhp"] = (np.float32(P_ZO) * whh_prep.astype(np.float32)
                 ).astype(np.float16)
    d["bg"] = bg.reshape(1, 4 * H)
    return d


def kernel(**inputs):
    if "nc" not in _CACHE:
        _CACHE["nc"] = _build()
    nc = _CACHE["nc"]
    in_maps = [_prep_core(inputs, c) for c in range(8)]
    res = run_bass_kernel_spmd(nc, in_maps, list(range(8)))
    _CACHE["last"] = res
    out = np.empty((B, T, 2 * H), np.float32)
    for c in range(8):
        bsl = slice(8 * (c % 4), 8 * (c % 4) + 8)
        arr = np.asarray(res.results[c]["out"], np.float32)
        arr = arr.reshape(T // RB, 128, RB, 2, BL)
        h = arr.transpose(4, 0, 2, 3, 1).reshape(BL, T, H)
        if c < 4:
            out[bsl, :, :H] = h
        else:
            out[bsl, :, H:] = h[:, ::-1, :]
    return out
